# revision 26
# baseline (speedup 1.0000x reference)
"""3-layer GCN encoder (GCNConv x4, layers 3+4 fused) on 8 Trainium2 NeuronCores.

Strategy (graph/data parallel, matches the edge-cut sharding hint):
  - Nodes are partitioned contiguously across the 8 cores (NLOC = N/8 per core).
  - Each layer: local transform H = h @ W (PE, fp32), rows scaled by dinv[node],
    cast to bf16, packed two-nodes-per-256B-row into a local table slice, then
    AllGather -> full table in each core's HBM.
  - Aggregation: per 128-dst-node window, dma_gather fetches the (src-pair) rows
    for all in-edges (dst-grouped, padded to x128); a one-hot matrix S built on
    DVE (iota-256 is_equal dst_rel + 128*parity) turns segment-sum into PE
    matmuls accumulating in PSUM (node-major [128,64] f32).
  - Epilogue: x dinv[dst], + bias, ReLU (layers 1,2); final layer writes
    [NLOC, 64] = [mu | logstd] to DRAM.
  - Pair packing keeps gather indices = src>>1 < 32768 (int16 limit), halves
    the exchanged table bytes, and the parity select folds into the one-hot.

Self-contained: only needs numpy/ml_dtypes/concourse (container-installed).
"""

import os
import sys

if "/opt/trn_rl_repo" not in sys.path:
    sys.path.insert(0, "/opt/trn_rl_repo")

import numpy as np
import ml_dtypes

import concourse.bass as bass
import concourse.bacc as bacc
import concourse.mybir as mybir
import concourse.tile as tile
from concourse.bass_utils import run_bass_kernel_spmd

BF16 = ml_dtypes.bfloat16
F32 = mybir.dt.float32
BF = mybir.dt.bfloat16
I16 = mybir.dt.int16

N_CORES = 8

_cache = {}
_last = {}


def last_run(trace=False, **kw):
    """Re-run the last compiled kernel/in_maps (optionally with NTFF tracing)."""
    if "nc" not in _last:
        return None
    return run_bass_kernel_spmd(_last["nc"], _last["in_maps"],
                                core_ids=list(range(N_CORES)), trace=trace, **kw)


def _prep_edges(src, dst, N, NLOC, W):
    """Group in-edges by (dst core, dst window); pad each window to x128 slots
    uniformly across cores. Returns per-core int16 gather indices / bf16 dsel
    tiles plus per-window padded counts."""
    Wp = W + (W & 1)          # windows padded to even (pair = adjacent windows)
    TA = (Wp // 2 + 1) // 2   # chunk-A t-range (src windows [0, 2*TA))
    TB = Wp // 2 - TA         # chunk-B t-range
    RA, RB = 128 * TA, 128 * TB
    core = dst // NLOC
    local = dst - core * NLOC
    w = local >> 7
    rel = local & 127
    # source -> (core, partition slot, window) -> chunk / row / parity
    sc = src // NLOC
    sl = src - sc * NLOC
    sp = sl & 127
    sw = sl >> 7
    st = sw >> 1
    spar = sw & 1
    isB = st >= TA
    srow = np.where(isB, sc * RB + sp * TB + (st - TA),
                    sc * RA + sp * TA + st)

    # group edges by (core, dst-window, chunk)
    key = (core * W + w) * 2 + isB
    order = np.argsort(key, kind="stable")
    ksort = key[order]
    counts = np.bincount(key, minlength=N_CORES * W * 2).reshape(N_CORES, W, 2)
    P_As = np.maximum((counts[:, :, 0].max(0) + 127) // 128 * 128, 128)
    if TB > 0:
        P_Bs = np.maximum((counts[:, :, 1].max(0) + 127) // 128 * 128, 128)
    else:
        P_Bs = np.zeros(W, np.int64)
    PA_tot = int(P_As.sum())
    # slot layout: [all A segments by window][all B segments by window]
    cumA = np.concatenate([[0], np.cumsum(P_As)])
    cumB = np.concatenate([[0], np.cumsum(P_Bs)]) + PA_tot
    P_tot = int(PA_tot + P_Bs.sum())
    gstart = np.concatenate([[0], np.cumsum(counts.reshape(-1))])
    pos_in_group = np.arange(len(ksort)) - gstart[ksort]
    w_of = (ksort >> 1) % W
    c_of = (ksort >> 1) // W
    b_of = ksort & 1
    slot = np.where(b_of == 1, cumB[w_of], cumA[w_of]) + pos_in_group

    idx_arr = np.zeros((N_CORES, P_tot), np.int16)
    dsel_arr = np.full((N_CORES, P_tot), 300.0, np.float32)
    idx_arr[c_of, slot] = srow[order].astype(np.int16)
    dsel_arr[c_of, slot] = rel[order] + 128.0 * spar[order]

    idx_tiles = []
    dsel_tiles = []
    for c in range(N_CORES):
        idx16 = np.ascontiguousarray(idx_arr[c].reshape(P_tot // 16, 16).T)
        idx_tiles.append(np.ascontiguousarray(np.tile(idx16, (8, 1))))
        dsel_tiles.append(
            np.ascontiguousarray(dsel_arr[c].reshape(P_tot // 128, 128).T)
        )
    return idx_tiles, dsel_tiles, (list(map(int, P_As)), list(map(int, P_Bs))), P_tot


def _build(N, NLOC, W, P_ABs):
    """Build the 8-core SPMD Bass program. Returns compiled nc."""
    P_As, P_Bs = [list(map(int, p)) for p in P_ABs]
    PA_tot = sum(P_As)
    P_tot = PA_tot + sum(P_Bs)
    NTA = [p // 128 for p in P_As]
    NTB = [p // 128 for p in P_Bs]
    NT_MAX = max(NTA + NTB)
    cumA = np.concatenate([[0], np.cumsum(P_As)]).astype(int)
    cumB = (np.concatenate([[0], np.cumsum(P_Bs)]) + PA_tot).astype(int)
    GRP = int(os.environ.get("K_GRP", "1"))  # windows per gather instruction

    solo = os.environ.get("K_SOLO", "0") == "1"
    nc = bacc.Bacc("TRN2", target_bir_lowering=False, debug=False,
                   num_devices=1 if solo else N_CORES)

    xT_d = nc.dram_tensor("xT", (128, NLOC), F32, kind="ExternalInput")
    idxs_d = nc.dram_tensor("idxs", (128, P_tot // 16), I16, kind="ExternalInput")
    dsel_d = nc.dram_tensor("dsel", (128, P_tot // 128), F32, kind="ExternalInput")
    degp_d = nc.dram_tensor("degp", (128, W), F32, kind="ExternalInput")
    W1_d = nc.dram_tensor("W1", (128, 64), F32, kind="ExternalInput")
    W2_d = nc.dram_tensor("W2", (64, 64), F32, kind="ExternalInput")
    W34_d = nc.dram_tensor("W34", (64, 64), F32, kind="ExternalInput")
    b1_d = nc.dram_tensor("b1b", (128, 64), F32, kind="ExternalInput")
    b2_d = nc.dram_tensor("b2b", (128, 64), F32, kind="ExternalInput")
    b34_d = nc.dram_tensor("b34b", (128, 64), F32, kind="ExternalInput")
    iota_d = nc.dram_tensor("iota", (128, 256), BF, kind="ExternalInput")
    id128_d = nc.dram_tensor("id128", (128, 128), F32, kind="ExternalInput")
    id64_d = nc.dram_tensor("id64", (64, 64), BF, kind="ExternalInput")
    out_d = nc.dram_tensor("out34", (NLOC, 64), F32, kind="ExternalOutput")

    Wp = W + (W & 1)
    TA = (Wp // 2 + 1) // 2
    TB = Wp // 2 - TA
    RA, RB = 128 * TA, 128 * TB
    tablA = [nc.dram_tensor(f"tablA{l}", (RA, 128), BF, kind="Internal")
             for l in range(3)]
    split = TB > 0
    tablB = [nc.dram_tensor(f"tablB{l}", (RB, 128), BF, kind="Internal")
             for l in range(3)] if split else None
    tabfA = [nc.dram_tensor(f"tabfA{l}", (N_CORES * RA, 128), BF, kind="Internal",
                            addr_space="Shared") for l in range(3)]
    tabfB = [nc.dram_tensor(f"tabfB{l}", (N_CORES * RB, 128), BF, kind="Internal",
                            addr_space="Shared") for l in range(3)] if split else None

    AG = mybir.AluOpType
    RG = [list(range(N_CORES))]

    with tile.TileContext(nc) as tc:
        with (
            tc.tile_pool(name="const", bufs=1) as const,
            tc.tile_pool(name="big", bufs=1) as big,
            tc.tile_pool(name="tt", bufs=1) as ttp,
            tc.tile_pool(name="work", bufs=4) as work,
            tc.tile_pool(name="gp", bufs=5) as gp,
            tc.tile_pool(name="sp", bufs=8) as sp,
            tc.tile_pool(name="psT", bufs=2, space="PSUM") as psT,
            tc.tile_pool(name="psR", bufs=2, space="PSUM") as psR,
            tc.tile_pool(name="psA", bufs=3, space="PSUM") as psA,
        ):
            # ---- constant loads ----
            def cload(dram, shape, dt, tag):
                t = const.tile(shape, dt, tag=tag)
                nc.sync.dma_start(t[:], dram[:])
                return t

            idxs = cload(idxs_d, [128, P_tot // 16], I16, "idxs")
            dsel = cload(dsel_d, [128, P_tot // 128], F32, "dsel")
            iota = cload(iota_d, [128, 256], BF, "iota")
            id128 = cload(id128_d, [128, 128], F32, "id128")
            id64 = cload(id64_d, [64, 64], BF, "id64")
            W1t = cload(W1_d, [128, 64], F32, "W1t")
            W2t = cload(W2_d, [64, 64], F32, "W2t")
            W34t = cload(W34_d, [64, 64], F32, "W34t")
            b1t = cload(b1_d, [128, 64], F32, "b1t")
            b2t = cload(b2_d, [128, 64], F32, "b2t")
            b34t = cload(b34_d, [128, 64], F32, "b34t")

            NCH = (NLOC + 511) // 512
            hT2 = [big.tile([64, 512], F32, name=f"hT2_{j}", tag=f"hT2_{j}")
                   for j in range(NCH)]
            hT3 = [big.tile([64, 512], F32, name=f"hT3_{j}", tag=f"hT3_{j}")
                   for j in range(NCH)]

            degp = const.tile([128, W], F32, tag="degp")
            nc.sync.dma_start(degp[:], degp_d[:])
            sqp = const.tile([128, W], F32, tag="sqp")
            nc.scalar.activation(sqp[:], degp[:], mybir.ActivationFunctionType.Sqrt)
            dinvp = const.tile([128, W], F32, tag="dinvp")
            nc.vector.reciprocal(dinvp[:], sqp[:])

            CP = int(os.environ.get("K_CP", "0"))

            def nw_cols(nw):
                return 64

            def transform(l, hT, K, Wt):
                """T^T = W^T @ hT, bf16 [64, NLOC] (dinv applied in build_table)."""
                TT = ttp.tile([64, NLOC], BF, tag="TT")
                for c0 in range(0, NLOC, 512):
                    cn = min(512, NLOC - c0)
                    if hT is None:
                        xc = work.tile([128, 512], F32, tag="xc")
                        nc.sync.dma_start(xc[:, :cn], xT_d[:, c0:c0 + cn])
                        rhs = xc[:K, :cn]
                    else:
                        rhs = hT[c0 // 512][:K, :cn]
                    ps = psT.tile([64, 512], F32, tag="psT")
                    nc.tensor.matmul(ps[:, :cn], Wt[:K, :], rhs,
                                     start=True, stop=True)
                    if CP & 2:
                        nc.vector.tensor_copy(TT[:, c0:c0 + cn], ps[:, :cn])
                    else:
                        nc.scalar.copy(TT[:, c0:c0 + cn], ps[:, :cn])
                return TT

            def build_table(l, TT):
                stage = ttp.tile([128, Wp * 64], BF, tag="stage")
                if W != Wp:
                    nc.vector.memset(stage[:, W * 64:], 0.0)
                for w in range(W):
                    c0 = 128 * w
                    nw = min(128, NLOC - c0)
                    ptt = psR.tile([128, 128], BF, tag="ptr")
                    pt = ptt[:, :64]
                    nc.tensor.transpose(pt[:nw, :], TT[:, c0:c0 + nw], id64[:, :])
                    if nw < 128:
                        nc.vector.memset(stage[:, 64 * w:64 * (w + 1)], 0.0)
                    nc.scalar.activation(
                        stage[:nw, 64 * w:64 * w + 64], pt[:nw, :],
                        mybir.ActivationFunctionType.Copy,
                        scale=dinvp[:nw, w:w + 1])
                CA = 2 * TA * 64
                nc.sync.dma_start(
                    tablA[l][:].rearrange("(p r) e -> p (r e)", p=128),
                    stage[:, :CA])
                if split:
                    nc.sync.dma_start(
                        tablB[l][:].rearrange("(p r) e -> p (r e)", p=128),
                        stage[:, CA:])
                if not solo:
                    nc.gpsimd.collective_compute(
                        "AllGather", AG.bypass, replica_groups=RG,
                        ins=[tablA[l][:].opt()], outs=[tabfA[l][:].opt()])
                    if split:
                        nc.gpsimd.collective_compute(
                            "AllGather", AG.bypass, replica_groups=RG,
                            ins=[tablB[l][:].opt()], outs=[tabfB[l][:].opt()])

            def aggregate(l, bias_t, relu, hT_next):
                partial = big.tile([128, W * 64], F32, name=f"partA{l}",
                                   tag="partA")

                def seg_pass(is_b):
                    last = is_b or not split
                    tabsrc = (tabfB if is_b else tabfA)[l]
                    nts = NTB if is_b else NTA
                    cums = cumB if is_b else cumA
                    pws = P_Bs if is_b else P_As
                    for w0 in range(0, W, GRP):
                        wn = min(GRP, W - w0)
                        ntg = sum(nts[w0:w0 + wn])
                        pg = int(cums[w0 + wn] - cums[w0])
                        soff = int(cums[w0]) // 16
                        g = gp.tile([128, GRP * NT_MAX, 128], BF, tag="g")
                        nc.gpsimd.dma_gather(
                            g[:, :ntg, :], tabsrc[:],
                            idxs[:, soff: soff + pg // 16],
                            pg, pg, 128, single_packet=False)
                        tb = 0
                        for w in range(w0, w0 + wn):
                            c0 = 128 * w
                            nw = min(128, NLOC - c0)
                            nt = nts[w]
                            toff = int(cums[w]) // 128
                            ps = psA.tile([128, 64], F32, tag="psA")
                            for t in range(nt):
                                S = sp.tile([128, 256], BF, tag="S")
                                nc.vector.tensor_scalar(
                                    out=S[:], in0=iota[:],
                                    scalar1=dsel[:, toff + t: toff + t + 1],
                                    scalar2=None, op0=AG.is_equal)
                                nc.tensor.matmul(
                                    ps[:], S[:, 0:128], g[:, tb + t, 0:64],
                                    start=(t == 0), stop=False)
                                nc.tensor.matmul(
                                    ps[:], S[:, 128:256], g[:, tb + t, 64:128],
                                    start=False, stop=(t == nt - 1))
                            tb += nt
                            if not last:
                                if CP & 1:
                                    nc.vector.tensor_copy(
                                        partial[:, 64 * w:64 * w + 64], ps[:])
                                else:
                                    nc.scalar.copy(
                                        partial[:, 64 * w:64 * w + 64], ps[:])
                                continue
                            # last pass epilogue: combine, scale, bias, relu
                            hw_ = work.tile([128, 64], F32, tag="hw")
                            if split:
                                nc.vector.tensor_tensor(
                                    out=hw_[:], in0=ps[:],
                                    in1=partial[:, 64 * w:64 * w + 64],
                                    op=AG.add)
                            else:
                                nc.scalar.copy(hw_[:], ps[:])
                            if CP & 8:
                                nc.vector.tensor_scalar(
                                    out=hw_[:], in0=hw_[:],
                                    scalar1=dinvp[:, w:w + 1],
                                    scalar2=None, op0=AG.mult)
                            else:
                                nc.scalar.activation(
                                    hw_[:], hw_[:],
                                    mybir.ActivationFunctionType.Copy,
                                    scale=dinvp[:, w:w + 1])
                            nc.vector.tensor_tensor(
                                out=hw_[:], in0=hw_[:], in1=bias_t[:], op=AG.add)
                            if relu:
                                nc.scalar.activation(
                                    hw_[:], hw_[:],
                                    mybir.ActivationFunctionType.Relu)
                            if hT_next is not None:
                                pt = psR.tile([64, 128], F32, tag="ptr")
                                nc.tensor.transpose(pt[:, :nw], hw_[:nw, :],
                                                    id128[:nw, :nw])
                                j, r0 = c0 // 512, c0 % 512
                                if CP & 4:
                                    nc.vector.tensor_copy(
                                        hT_next[j][:, r0:r0 + nw], pt[:, :nw])
                                else:
                                    nc.scalar.copy(hT_next[j][:, r0:r0 + nw],
                                                   pt[:, :nw])
                            else:
                                nc.sync.dma_start(out_d[c0:c0 + nw, :],
                                                  hw_[:nw, :])

                seg_pass(False)
                if split:
                    seg_pass(True)

            PH = int(os.environ.get("K_PHASES", "9"))
            ONLY_AGG = os.environ.get("K_ONLY_AGG", "0") == "1"
            if ONLY_AGG:
                for _l in range(3):
                    aggregate(0, b1t, True, hT2)
                PH = 0
            REP = int(os.environ.get("K_REPEAT", "1"))
            for _rep in range(REP):
                if PH >= 1:
                    TT = transform(0, None, 128, W1t)
                if PH >= 2:
                    build_table(0, TT)
                if PH >= 3:
                    aggregate(0, b1t, True, hT2)
                if PH >= 4:
                    TT = transform(1, hT2, 64, W2t)
                if PH >= 5:
                    build_table(1, TT)
                if PH >= 6:
                    aggregate(1, b2t, True, hT3)
                if PH >= 7:
                    TT = transform(2, hT3, 64, W34t)
                if PH >= 8:
                    build_table(2, TT)
                if PH >= 9:
                    aggregate(2, b34t, False, None)

    nc.compile()
    return nc


def _run(inputs, N, E):
    NLOC = N // N_CORES
    W = (NLOC + 127) // 128

    x = np.asarray(inputs["x"], np.float32)
    ei = np.asarray(inputs["edge_index"], np.int64)
    W1 = np.asarray(inputs["W1"], np.float32)
    b1 = np.asarray(inputs["b1"], np.float32)
    W2 = np.asarray(inputs["W2"], np.float32)
    b2 = np.asarray(inputs["b2"], np.float32)
    Wmu = np.asarray(inputs["Wmu"], np.float32)
    bmu = np.asarray(inputs["bmu"], np.float32)
    Wls = np.asarray(inputs["Wls"], np.float32)
    bls = np.asarray(inputs["bls"], np.float32)

    loop = np.arange(N, dtype=np.int64)
    src = np.concatenate([ei[0], loop])
    dst = np.concatenate([ei[1], loop])
    deg = np.bincount(dst, minlength=N).astype(np.float32)

    idx_tiles, dsel_tiles, P_ws, P_tot = _prep_edges(src, dst, N, NLOC, W)

    key = (N, NLOC, W, tuple(P_ws[0]), tuple(P_ws[1]),
           os.environ.get("K_PHASES", "9"), os.environ.get("K_REPEAT", "1"),
           os.environ.get("K_SOLO", "0"), os.environ.get("K_ONLY_AGG", "0"),
           os.environ.get("K_CP", "0"), os.environ.get("K_GRP", "4"))
    if key not in _cache:
        _cache[key] = _build(N, NLOC, W, P_ws)
    nc = _cache[key]

    W34 = np.concatenate([Wmu, Wls], axis=1)
    b34 = np.concatenate([bmu, bls])
    iota = np.ascontiguousarray(np.tile(np.arange(256, dtype=np.float32),
                                        (128, 1))).astype(BF16)
    id128 = np.eye(128, dtype=np.float32)
    id64 = np.eye(64, dtype=np.float32).astype(BF16)
    b1b = np.ascontiguousarray(np.tile(b1, (128, 1)))
    b2b = np.ascontiguousarray(np.tile(b2, (128, 1)))
    b34b = np.ascontiguousarray(np.tile(b34, (128, 1)))

    in_maps = []
    for c in range(N_CORES):
        degc = deg[c * NLOC:(c + 1) * NLOC]
        degp = np.ones(W * 128, np.float32)
        degp[:NLOC] = degc
        in_maps.append({
            "xT": np.ascontiguousarray(x[c * NLOC:(c + 1) * NLOC].T),
            "idxs": idx_tiles[c],
            "dsel": dsel_tiles[c],
            "degp": np.ascontiguousarray(degp.reshape(W, 128).T),
            "W1": W1, "W2": W2, "W34": W34,
            "b1b": b1b, "b2b": b2b, "b34b": b34b,
            "iota": iota, "id128": id128, "id64": id64,
        })

    _last["nc"] = nc
    _last["in_maps"] = in_maps
    res = run_bass_kernel_spmd(nc, in_maps, core_ids=list(range(N_CORES)))
    out = np.concatenate([res.results[c]["out34"] for c in range(N_CORES)], axis=0)
    return out[:, :32].copy(), out[:, 32:].copy()


def kernel(**inputs):
    x = np.asarray(inputs["x"])
    ei = np.asarray(inputs["edge_index"])
    return _run(inputs, x.shape[0], ei.shape[1])


# revision 29
# speedup vs baseline: 1.0191x; 1.0191x over previous
"""3-layer GCN encoder (GCNConv x4, layers 3+4 fused) on 8 Trainium2 NeuronCores.

Strategy (graph/data parallel, matches the edge-cut sharding hint):
  - Nodes are partitioned contiguously across the 8 cores (NLOC = N/8 per core).
  - Each layer: local transform H = h @ W (PE, fp32), rows scaled by dinv[node],
    cast to bf16, packed two-nodes-per-256B-row into a local table slice, then
    AllGather -> full table in each core's HBM.
  - Aggregation: per 128-dst-node window, dma_gather fetches the (src-pair) rows
    for all in-edges (dst-grouped, padded to x128); a one-hot matrix S built on
    DVE (iota-256 is_equal dst_rel + 128*parity) turns segment-sum into PE
    matmuls accumulating in PSUM (node-major [128,64] f32).
  - Epilogue: x dinv[dst], + bias, ReLU (layers 1,2); final layer writes
    [NLOC, 64] = [mu | logstd] to DRAM.
  - Pair packing keeps gather indices = src>>1 < 32768 (int16 limit), halves
    the exchanged table bytes, and the parity select folds into the one-hot.

Self-contained: only needs numpy/ml_dtypes/concourse (container-installed).
"""

import os
import sys

if "/opt/trn_rl_repo" not in sys.path:
    sys.path.insert(0, "/opt/trn_rl_repo")

import numpy as np
import ml_dtypes

import concourse.bass as bass
import concourse.bacc as bacc
import concourse.mybir as mybir
import concourse.tile as tile
from concourse.bass_utils import run_bass_kernel_spmd

BF16 = ml_dtypes.bfloat16
F32 = mybir.dt.float32
BF = mybir.dt.bfloat16
I16 = mybir.dt.int16

N_CORES = 8

_cache = {}
_last = {}


def last_run(trace=False, **kw):
    """Re-run the last compiled kernel/in_maps (optionally with NTFF tracing)."""
    if "nc" not in _last:
        return None
    return run_bass_kernel_spmd(_last["nc"], _last["in_maps"],
                                core_ids=list(range(N_CORES)), trace=trace, **kw)


def _prep_edges(src, dst, N, NLOC, W, lpos=None):
    """Group in-edges by (dst core, dst window); pad each window to x128 slots
    uniformly across cores. Returns per-core int16 gather indices / bf16 dsel
    tiles plus per-window padded counts."""
    Wp = W + (W & 1)          # windows padded to even (pair = adjacent windows)
    TA = (Wp // 2 + 1) // 2   # chunk-A t-range (src windows [0, 2*TA))
    TB = Wp // 2 - TA         # chunk-B t-range
    RA, RB = 128 * TA, 128 * TB
    if lpos is None:
        lpos = np.arange(len(np.empty(0)))  # placeholder
        lpos = np.arange(N, dtype=np.int64) % NLOC
    core = dst // NLOC
    local = lpos[dst]
    w = local >> 7
    rel = local & 127
    # source -> (core, partition slot, window) -> chunk / row / parity
    sc = src // NLOC
    sl = lpos[src]
    sp = sl & 127
    sw = sl >> 7
    st = sw >> 1
    spar = sw & 1
    isB = st >= TA
    srow = np.where(isB, sc * RB + sp * TB + (st - TA),
                    sc * RA + sp * TA + st)

    # group edges by (core, dst-window, chunk)
    key = (core * W + w) * 2 + isB
    order = np.argsort(key, kind="stable")
    ksort = key[order]
    counts = np.bincount(key, minlength=N_CORES * W * 2).reshape(N_CORES, W, 2)
    P_As = np.maximum((counts[:, :, 0].max(0) + 127) // 128 * 128, 128)
    if TB > 0:
        P_Bs = np.maximum((counts[:, :, 1].max(0) + 127) // 128 * 128, 128)
    else:
        P_Bs = np.zeros(W, np.int64)
    PA_tot = int(P_As.sum())
    # slot layout: [all A segments by window][all B segments by window]
    cumA = np.concatenate([[0], np.cumsum(P_As)])
    cumB = np.concatenate([[0], np.cumsum(P_Bs)]) + PA_tot
    P_tot = int(PA_tot + P_Bs.sum())
    gstart = np.concatenate([[0], np.cumsum(counts.reshape(-1))])
    pos_in_group = np.arange(len(ksort)) - gstart[ksort]
    w_of = (ksort >> 1) % W
    c_of = (ksort >> 1) // W
    b_of = ksort & 1
    slot = np.where(b_of == 1, cumB[w_of], cumA[w_of]) + pos_in_group

    idx_arr = np.zeros((N_CORES, P_tot), np.int16)
    dsel_arr = np.full((N_CORES, P_tot), 300.0, np.float32)
    idx_arr[c_of, slot] = srow[order].astype(np.int16)
    dsel_arr[c_of, slot] = rel[order] + 128.0 * spar[order]

    idx_tiles = []
    dsel_tiles = []
    for c in range(N_CORES):
        idx16 = np.ascontiguousarray(idx_arr[c].reshape(P_tot // 16, 16).T)
        idx_tiles.append(np.ascontiguousarray(np.tile(idx16, (8, 1))))
        dsel_tiles.append(
            np.ascontiguousarray(dsel_arr[c].reshape(P_tot // 128, 128).T)
        )
    return idx_tiles, dsel_tiles, (list(map(int, P_As)), list(map(int, P_Bs))), P_tot


def _build(N, NLOC, W, P_ABs):
    """Build the 8-core SPMD Bass program. Returns compiled nc."""
    P_As, P_Bs = [list(map(int, p)) for p in P_ABs]
    PA_tot = sum(P_As)
    P_tot = PA_tot + sum(P_Bs)
    NTA = [p // 128 for p in P_As]
    NTB = [p // 128 for p in P_Bs]
    NT_MAX = max(NTA + NTB)
    cumA = np.concatenate([[0], np.cumsum(P_As)]).astype(int)
    cumB = (np.concatenate([[0], np.cumsum(P_Bs)]) + PA_tot).astype(int)
    GRP = int(os.environ.get("K_GRP", "1"))  # windows per gather instruction

    solo = os.environ.get("K_SOLO", "0") == "1"
    nc = bacc.Bacc("TRN2", target_bir_lowering=False, debug=False,
                   num_devices=1 if solo else N_CORES)

    xT_d = nc.dram_tensor("xT", (128, NLOC), F32, kind="ExternalInput")
    idxs_d = nc.dram_tensor("idxs", (128, P_tot // 16), I16, kind="ExternalInput")
    dsel_d = nc.dram_tensor("dsel", (128, P_tot // 128), F32, kind="ExternalInput")
    degp_d = nc.dram_tensor("degp", (128, W), F32, kind="ExternalInput")
    W1_d = nc.dram_tensor("W1", (128, 64), F32, kind="ExternalInput")
    W2_d = nc.dram_tensor("W2", (64, 64), F32, kind="ExternalInput")
    W34_d = nc.dram_tensor("W34", (64, 64), F32, kind="ExternalInput")
    b1_d = nc.dram_tensor("b1b", (128, 64), F32, kind="ExternalInput")
    b2_d = nc.dram_tensor("b2b", (128, 64), F32, kind="ExternalInput")
    b34_d = nc.dram_tensor("b34b", (128, 64), F32, kind="ExternalInput")
    iota_d = nc.dram_tensor("iota", (128, 256), BF, kind="ExternalInput")
    id128_d = nc.dram_tensor("id128", (128, 128), F32, kind="ExternalInput")
    id64_d = nc.dram_tensor("id64", (64, 64), BF, kind="ExternalInput")
    out_d = nc.dram_tensor("out34", (NLOC, 64), F32, kind="ExternalOutput")

    Wp = W + (W & 1)
    TA = (Wp // 2 + 1) // 2
    TB = Wp // 2 - TA
    RA, RB = 128 * TA, 128 * TB
    tablA = [nc.dram_tensor(f"tablA{l}", (RA, 128), BF, kind="Internal")
             for l in range(3)]
    split = TB > 0
    tablB = [nc.dram_tensor(f"tablB{l}", (RB, 128), BF, kind="Internal")
             for l in range(3)] if split else None
    tabfA = [nc.dram_tensor(f"tabfA{l}", (N_CORES * RA, 128), BF, kind="Internal",
                            addr_space="Shared") for l in range(3)]
    tabfB = [nc.dram_tensor(f"tabfB{l}", (N_CORES * RB, 128), BF, kind="Internal",
                            addr_space="Shared") for l in range(3)] if split else None

    AG = mybir.AluOpType
    RG = [list(range(N_CORES))]

    with tile.TileContext(nc) as tc:
        with (
            tc.tile_pool(name="const", bufs=1) as const,
            tc.tile_pool(name="big", bufs=1) as big,
            tc.tile_pool(name="tt", bufs=1) as ttp,
            tc.tile_pool(name="work", bufs=4) as work,
            tc.tile_pool(name="gp", bufs=int(os.environ.get("K_GP","8"))) as gp,
            tc.tile_pool(name="sp", bufs=8) as sp,
            tc.tile_pool(name="psT", bufs=2, space="PSUM") as psT,
            tc.tile_pool(name="psR", bufs=2, space="PSUM") as psR,
            tc.tile_pool(name="psA", bufs=int(os.environ.get("K_PSA","4")), space="PSUM") as psA,
        ):
            # ---- constant loads ----
            def cload(dram, shape, dt, tag):
                t = const.tile(shape, dt, tag=tag)
                nc.sync.dma_start(t[:], dram[:])
                return t

            idxs = cload(idxs_d, [128, P_tot // 16], I16, "idxs")
            dsel = cload(dsel_d, [128, P_tot // 128], F32, "dsel")
            iota = cload(iota_d, [128, 256], BF, "iota")
            id128 = cload(id128_d, [128, 128], F32, "id128")
            id64 = cload(id64_d, [64, 64], BF, "id64")
            W1t = cload(W1_d, [128, 64], F32, "W1t")
            W2t = cload(W2_d, [64, 64], F32, "W2t")
            W34t = cload(W34_d, [64, 64], F32, "W34t")
            b1t = cload(b1_d, [128, 64], F32, "b1t")
            b2t = cload(b2_d, [128, 64], F32, "b2t")
            b34t = cload(b34_d, [128, 64], F32, "b34t")

            NCH = (NLOC + 511) // 512
            hT2 = [big.tile([64, 512], F32, name=f"hT2_{j}", tag=f"hT2_{j}")
                   for j in range(NCH)]
            hT3 = [big.tile([64, 512], F32, name=f"hT3_{j}", tag=f"hT3_{j}")
                   for j in range(NCH)]

            degp = const.tile([128, W], F32, tag="degp")
            nc.sync.dma_start(degp[:], degp_d[:])
            sqp = const.tile([128, W], F32, tag="sqp")
            nc.scalar.activation(sqp[:], degp[:], mybir.ActivationFunctionType.Sqrt)
            dinvp = const.tile([128, W], F32, tag="dinvp")
            nc.vector.reciprocal(dinvp[:], sqp[:])

            CP = int(os.environ.get("K_CP", "0"))

            def nw_cols(nw):
                return 64

            def transform(l, hT, K, Wt):
                """T^T = W^T @ hT, bf16 [64, NLOC] (dinv applied in build_table)."""
                TT = ttp.tile([64, NLOC], BF, tag="TT")
                for c0 in range(0, NLOC, 512):
                    cn = min(512, NLOC - c0)
                    if hT is None:
                        xc = work.tile([128, 512], F32, tag="xc")
                        nc.sync.dma_start(xc[:, :cn], xT_d[:, c0:c0 + cn])
                        rhs = xc[:K, :cn]
                    else:
                        rhs = hT[c0 // 512][:K, :cn]
                    ps = psT.tile([64, 512], F32, tag="psT")
                    nc.tensor.matmul(ps[:, :cn], Wt[:K, :], rhs,
                                     start=True, stop=True)
                    if CP & 2:
                        nc.vector.tensor_copy(TT[:, c0:c0 + cn], ps[:, :cn])
                    else:
                        nc.scalar.copy(TT[:, c0:c0 + cn], ps[:, :cn])
                return TT

            def build_table(l, TT):
                stage = ttp.tile([128, Wp * 64], BF, tag="stage")
                if W != Wp:
                    nc.vector.memset(stage[:, W * 64:], 0.0)
                for w in range(W):
                    c0 = 128 * w
                    nw = min(128, NLOC - c0)
                    ptt = psR.tile([128, 128], BF, tag="ptr")
                    pt = ptt[:, :64]
                    nc.tensor.transpose(pt[:nw, :], TT[:, c0:c0 + nw], id64[:, :])
                    if nw < 128:
                        nc.vector.memset(stage[:, 64 * w:64 * (w + 1)], 0.0)
                    nc.scalar.activation(
                        stage[:nw, 64 * w:64 * w + 64], pt[:nw, :],
                        mybir.ActivationFunctionType.Copy,
                        scale=dinvp[:nw, w:w + 1])
                CA = 2 * TA * 64
                nc.sync.dma_start(
                    tablA[l][:].rearrange("(p r) e -> p (r e)", p=128),
                    stage[:, :CA])
                if split:
                    nc.sync.dma_start(
                        tablB[l][:].rearrange("(p r) e -> p (r e)", p=128),
                        stage[:, CA:])
                if not solo:
                    nc.gpsimd.collective_compute(
                        "AllGather", AG.bypass, replica_groups=RG,
                        ins=[tablA[l][:].opt()], outs=[tabfA[l][:].opt()])
                    if split:
                        nc.gpsimd.collective_compute(
                            "AllGather", AG.bypass, replica_groups=RG,
                            ins=[tablB[l][:].opt()], outs=[tabfB[l][:].opt()])

            def aggregate(l, bias_t, relu, hT_next):
                partial = big.tile([128, W * 64], F32, name=f"partA{l}",
                                   tag="partA")

                def seg_pass(is_b):
                    last = is_b or not split
                    tabsrc = (tabfB if is_b else tabfA)[l]
                    nts = NTB if is_b else NTA
                    cums = cumB if is_b else cumA
                    pws = P_Bs if is_b else P_As
                    for w0 in range(0, W, GRP):
                        wn = min(GRP, W - w0)
                        ntg = sum(nts[w0:w0 + wn])
                        pg = int(cums[w0 + wn] - cums[w0])
                        soff = int(cums[w0]) // 16
                        g = gp.tile([128, GRP * NT_MAX, 128], BF, tag="g")
                        nc.gpsimd.dma_gather(
                            g[:, :ntg, :], tabsrc[:],
                            idxs[:, soff: soff + pg // 16],
                            pg, pg, 128, single_packet=False)
                        tb = 0
                        for w in range(w0, w0 + wn):
                            c0 = 128 * w
                            nw = min(128, NLOC - c0)
                            nt = nts[w]
                            toff = int(cums[w]) // 128
                            ps = psA.tile([128, 64], F32, tag="psA")
                            for t in range(nt):
                                S = sp.tile([128, 256], BF, tag="S")
                                nc.vector.tensor_scalar(
                                    out=S[:], in0=iota[:],
                                    scalar1=dsel[:, toff + t: toff + t + 1],
                                    scalar2=None, op0=AG.is_equal)
                                nc.tensor.matmul(
                                    ps[:], S[:, 0:128], g[:, tb + t, 0:64],
                                    start=(t == 0), stop=False)
                                nc.tensor.matmul(
                                    ps[:], S[:, 128:256], g[:, tb + t, 64:128],
                                    start=False, stop=(t == nt - 1))
                            tb += nt
                            if not last:
                                if CP & 1:
                                    nc.vector.tensor_copy(
                                        partial[:, 64 * w:64 * w + 64], ps[:])
                                else:
                                    nc.scalar.copy(
                                        partial[:, 64 * w:64 * w + 64], ps[:])
                                continue
                            # last pass epilogue: combine, scale, bias, relu
                            hw_ = work.tile([128, 64], F32, tag="hw")
                            if split:
                                nc.vector.tensor_tensor(
                                    out=hw_[:], in0=ps[:],
                                    in1=partial[:, 64 * w:64 * w + 64],
                                    op=AG.add)
                            else:
                                nc.scalar.copy(hw_[:], ps[:])
                            if CP & 8:
                                nc.vector.tensor_scalar(
                                    out=hw_[:], in0=hw_[:],
                                    scalar1=dinvp[:, w:w + 1],
                                    scalar2=None, op0=AG.mult)
                            else:
                                nc.scalar.activation(
                                    hw_[:], hw_[:],
                                    mybir.ActivationFunctionType.Copy,
                                    scale=dinvp[:, w:w + 1])
                            nc.vector.tensor_tensor(
                                out=hw_[:], in0=hw_[:], in1=bias_t[:], op=AG.add)
                            if relu:
                                nc.scalar.activation(
                                    hw_[:], hw_[:],
                                    mybir.ActivationFunctionType.Relu)
                            if hT_next is not None:
                                pt = psR.tile([64, 128], F32, tag="ptr")
                                nc.tensor.transpose(pt[:, :nw], hw_[:nw, :],
                                                    id128[:nw, :nw])
                                j, r0 = c0 // 512, c0 % 512
                                if CP & 4:
                                    nc.vector.tensor_copy(
                                        hT_next[j][:, r0:r0 + nw], pt[:, :nw])
                                else:
                                    nc.scalar.copy(hT_next[j][:, r0:r0 + nw],
                                                   pt[:, :nw])
                            else:
                                nc.sync.dma_start(out_d[c0:c0 + nw, :],
                                                  hw_[:nw, :])

                seg_pass(False)
                if split:
                    seg_pass(True)

            PH = int(os.environ.get("K_PHASES", "9"))
            ONLY_AGG = os.environ.get("K_ONLY_AGG", "0") == "1"
            if ONLY_AGG:
                for _l in range(3):
                    aggregate(0, b1t, True, hT2)
                PH = 0
            REP = int(os.environ.get("K_REPEAT", "1"))
            for _rep in range(REP):
                if PH >= 1:
                    TT = transform(0, None, 128, W1t)
                if PH >= 2:
                    build_table(0, TT)
                if PH >= 3:
                    aggregate(0, b1t, True, hT2)
                if PH >= 4:
                    TT = transform(1, hT2, 64, W2t)
                if PH >= 5:
                    build_table(1, TT)
                if PH >= 6:
                    aggregate(1, b2t, True, hT3)
                if PH >= 7:
                    TT = transform(2, hT3, 64, W34t)
                if PH >= 8:
                    build_table(2, TT)
                if PH >= 9:
                    aggregate(2, b34t, False, None)

    nc.compile()
    return nc


def _run(inputs, N, E):
    NLOC = N // N_CORES
    W = (NLOC + 127) // 128

    x = np.asarray(inputs["x"], np.float32)
    ei = np.asarray(inputs["edge_index"], np.int64)
    W1 = np.asarray(inputs["W1"], np.float32)
    b1 = np.asarray(inputs["b1"], np.float32)
    W2 = np.asarray(inputs["W2"], np.float32)
    b2 = np.asarray(inputs["b2"], np.float32)
    Wmu = np.asarray(inputs["Wmu"], np.float32)
    bmu = np.asarray(inputs["bmu"], np.float32)
    Wls = np.asarray(inputs["Wls"], np.float32)
    bls = np.asarray(inputs["bls"], np.float32)

    loop = np.arange(N, dtype=np.int64)
    src = np.concatenate([ei[0], loop])
    dst = np.concatenate([ei[1], loop])
    deg = np.bincount(dst, minlength=N).astype(np.float32)

    # balanced window assignment: deal degree-sorted nodes round-robin into
    # windows (equalizes per-window edge counts across cores -> less padding)
    lpos = np.empty(N, np.int64)
    caps = np.full(W, 128, np.int64)
    caps[W - 1] = NLOC - 128 * (W - 1)
    for c in range(N_CORES):
        dl = deg[c * NLOC:(c + 1) * NLOC]
        order_ = np.argsort(-dl, kind="stable")
        fill = np.zeros(W, np.int64)
        wi = 0
        pos = np.empty(NLOC, np.int64)
        for i in range(NLOC):
            while fill[wi % W] >= caps[wi % W]:
                wi += 1
            ww = wi % W
            pos[order_[i]] = ww * 128 + fill[ww]
            fill[ww] += 1
            wi += 1
        lpos[c * NLOC:(c + 1) * NLOC] = pos

    idx_tiles, dsel_tiles, P_ws, P_tot = _prep_edges(src, dst, N, NLOC, W, lpos)

    key = (N, NLOC, W, tuple(P_ws[0]), tuple(P_ws[1]),
           os.environ.get("K_PHASES", "9"), os.environ.get("K_REPEAT", "1"),
           os.environ.get("K_SOLO", "0"), os.environ.get("K_ONLY_AGG", "0"),
           os.environ.get("K_CP", "0"), os.environ.get("K_GRP", "4"))
    if key not in _cache:
        _cache[key] = _build(N, NLOC, W, P_ws)
    nc = _cache[key]

    W34 = np.concatenate([Wmu, Wls], axis=1)
    b34 = np.concatenate([bmu, bls])
    iota = np.ascontiguousarray(np.tile(np.arange(256, dtype=np.float32),
                                        (128, 1))).astype(BF16)
    id128 = np.eye(128, dtype=np.float32)
    id64 = np.eye(64, dtype=np.float32).astype(BF16)
    b1b = np.ascontiguousarray(np.tile(b1, (128, 1)))
    b2b = np.ascontiguousarray(np.tile(b2, (128, 1)))
    b34b = np.ascontiguousarray(np.tile(b34, (128, 1)))

    in_maps = []
    for c in range(N_CORES):
        degc = deg[c * NLOC:(c + 1) * NLOC]
        lc = lpos[c * NLOC:(c + 1) * NLOC]
        degp = np.ones(W * 128, np.float32)
        degp[lc] = degc
        xp = np.empty((NLOC, x.shape[1]), np.float32)
        xp[lc] = x[c * NLOC:(c + 1) * NLOC]
        in_maps.append({
            "xT": np.ascontiguousarray(xp.T),
            "idxs": idx_tiles[c],
            "dsel": dsel_tiles[c],
            "degp": np.ascontiguousarray(degp.reshape(W, 128).T),
            "W1": W1, "W2": W2, "W34": W34,
            "b1b": b1b, "b2b": b2b, "b34b": b34b,
            "iota": iota, "id128": id128, "id64": id64,
        })

    _last["nc"] = nc
    _last["in_maps"] = in_maps
    res = run_bass_kernel_spmd(nc, in_maps, core_ids=list(range(N_CORES)))
    out = np.empty((N, 64), np.float32)
    for c in range(N_CORES):
        lc = lpos[c * NLOC:(c + 1) * NLOC]
        out[c * NLOC:(c + 1) * NLOC] = res.results[c]["out34"][lc]
    return out[:, :32].copy(), out[:, 32:].copy()


def kernel(**inputs):
    x = np.asarray(inputs["x"])
    ei = np.asarray(inputs["edge_index"])
    return _run(inputs, x.shape[0], ei.shape[1])


# revision 30
# speedup vs baseline: 1.0372x; 1.0178x over previous
"""3-layer GCN encoder (GCNConv x4, layers 3+4 fused) on 8 Trainium2 NeuronCores.

Strategy (graph/data parallel, matches the edge-cut sharding hint):
  - Nodes are partitioned contiguously across the 8 cores (NLOC = N/8 per core).
  - Each layer: local transform H = h @ W (PE, fp32), rows scaled by dinv[node],
    cast to bf16, packed two-nodes-per-256B-row into a local table slice, then
    AllGather -> full table in each core's HBM.
  - Aggregation: per 128-dst-node window, dma_gather fetches the (src-pair) rows
    for all in-edges (dst-grouped, padded to x128); a one-hot matrix S built on
    DVE (iota-256 is_equal dst_rel + 128*parity) turns segment-sum into PE
    matmuls accumulating in PSUM (node-major [128,64] f32).
  - Epilogue: x dinv[dst], + bias, ReLU (layers 1,2); final layer writes
    [NLOC, 64] = [mu | logstd] to DRAM.
  - Pair packing keeps gather indices = src>>1 < 32768 (int16 limit), halves
    the exchanged table bytes, and the parity select folds into the one-hot.

Self-contained: only needs numpy/ml_dtypes/concourse (container-installed).
"""

import os
import sys

if "/opt/trn_rl_repo" not in sys.path:
    sys.path.insert(0, "/opt/trn_rl_repo")

import numpy as np
import ml_dtypes

import concourse.bass as bass
import concourse.bacc as bacc
import concourse.mybir as mybir
import concourse.tile as tile
from concourse.bass_utils import run_bass_kernel_spmd

BF16 = ml_dtypes.bfloat16
F32 = mybir.dt.float32
BF = mybir.dt.bfloat16
I16 = mybir.dt.int16

N_CORES = 8

_cache = {}
_last = {}


def last_run(trace=False, **kw):
    """Re-run the last compiled kernel/in_maps (optionally with NTFF tracing)."""
    if "nc" not in _last:
        return None
    return run_bass_kernel_spmd(_last["nc"], _last["in_maps"],
                                core_ids=list(range(N_CORES)), trace=trace, **kw)


def _balance(deg, N, NLOC, W):
    """Deal degree-sorted nodes round-robin into windows, per core."""
    lpos = np.empty(N, np.int64)
    caps = np.full(W, 128, np.int64)
    caps[W - 1] = NLOC - 128 * (W - 1)
    for c in range(N_CORES):
        dl = deg[c * NLOC:(c + 1) * NLOC]
        order_ = np.argsort(-dl, kind="stable")
        fill = np.zeros(W, np.int64)
        wi = 0
        pos = np.empty(NLOC, np.int64)
        for i in range(NLOC):
            while fill[wi % W] >= caps[wi % W]:
                wi += 1
            ww = wi % W
            pos[order_[i]] = ww * 128 + fill[ww]
            fill[ww] += 1
            wi += 1
        lpos[c * NLOC:(c + 1) * NLOC] = pos
    return lpos


def _prep_edges(src, dst, N, NLOC, W, lpos=None):
    """Group in-edges by (dst core, dst window); pad each window to x128 slots
    uniformly across cores. Returns per-core int16 gather indices / bf16 dsel
    tiles plus per-window padded counts."""
    Wp = W + (W & 1)          # windows padded to even (pair = adjacent windows)
    TA = (Wp // 2 + 1) // 2   # chunk-A t-range (src windows [0, 2*TA))
    TB = Wp // 2 - TA         # chunk-B t-range
    RA, RB = 128 * TA, 128 * TB
    if lpos is None:
        lpos = np.arange(len(np.empty(0)))  # placeholder
        lpos = np.arange(N, dtype=np.int64) % NLOC
    core = dst // NLOC
    local = lpos[dst]
    w = local >> 7
    rel = local & 127
    # source -> (core, partition slot, window) -> chunk / row / parity
    sc = src // NLOC
    sl = lpos[src]
    sp = sl & 127
    sw = sl >> 7
    st = sw >> 1
    spar = sw & 1
    isB = st >= TA
    srow = np.where(isB, sc * RB + sp * TB + (st - TA),
                    sc * RA + sp * TA + st)

    # group edges by (core, dst-window, chunk)
    key = (core * W + w) * 2 + isB
    order = np.argsort(key, kind="stable")
    ksort = key[order]
    counts = np.bincount(key, minlength=N_CORES * W * 2).reshape(N_CORES, W, 2)
    P_As = np.maximum((counts[:, :, 0].max(0) + 127) // 128 * 128, 128)
    if TB > 0:
        P_Bs = np.maximum((counts[:, :, 1].max(0) + 127) // 128 * 128, 128)
    else:
        P_Bs = np.zeros(W, np.int64)
    PA_tot = int(P_As.sum())
    # slot layout: [all A segments by window][all B segments by window]
    cumA = np.concatenate([[0], np.cumsum(P_As)])
    cumB = np.concatenate([[0], np.cumsum(P_Bs)]) + PA_tot
    P_tot = int(PA_tot + P_Bs.sum())
    gstart = np.concatenate([[0], np.cumsum(counts.reshape(-1))])
    pos_in_group = np.arange(len(ksort)) - gstart[ksort]
    w_of = (ksort >> 1) % W
    c_of = (ksort >> 1) // W
    b_of = ksort & 1
    slot = np.where(b_of == 1, cumB[w_of], cumA[w_of]) + pos_in_group

    idx_arr = np.zeros((N_CORES, P_tot), np.int16)
    dsel_arr = np.full((N_CORES, P_tot), 300.0, np.float32)
    idx_arr[c_of, slot] = srow[order].astype(np.int16)
    dsel_arr[c_of, slot] = rel[order] + 128.0 * spar[order]

    idx_tiles = []
    dsel_tiles = []
    for c in range(N_CORES):
        idx16 = np.ascontiguousarray(idx_arr[c].reshape(P_tot // 16, 16).T)
        idx_tiles.append(np.ascontiguousarray(np.tile(idx16, (8, 1))))
        dsel_tiles.append(
            np.ascontiguousarray(dsel_arr[c].reshape(P_tot // 128, 128).T)
        )
    return idx_tiles, dsel_tiles, (list(map(int, P_As)), list(map(int, P_Bs))), P_tot


def _build(N, NLOC, W, P_ABs):
    """Build the 8-core SPMD Bass program. Returns compiled nc."""
    P_As, P_Bs = [list(map(int, p)) for p in P_ABs]
    PA_tot = sum(P_As)
    P_tot = PA_tot + sum(P_Bs)
    NTA = [p // 128 for p in P_As]
    NTB = [p // 128 for p in P_Bs]
    NT_MAX = max(NTA + NTB)
    cumA = np.concatenate([[0], np.cumsum(P_As)]).astype(int)
    cumB = (np.concatenate([[0], np.cumsum(P_Bs)]) + PA_tot).astype(int)
    GRP = int(os.environ.get("K_GRP", "1"))  # windows per gather instruction

    solo = os.environ.get("K_SOLO", "0") == "1"
    nc = bacc.Bacc("TRN2", target_bir_lowering=False, debug=False,
                   num_devices=1 if solo else N_CORES)

    xT_d = nc.dram_tensor("xT", (128, NLOC), F32, kind="ExternalInput")
    idxs_d = nc.dram_tensor("idxs", (128, P_tot // 16), I16, kind="ExternalInput")
    dsel_d = nc.dram_tensor("dsel", (128, P_tot // 128), F32, kind="ExternalInput")
    degp_d = nc.dram_tensor("degp", (128, W), F32, kind="ExternalInput")
    W1_d = nc.dram_tensor("W1", (128, 64), F32, kind="ExternalInput")
    W2_d = nc.dram_tensor("W2", (64, 64), F32, kind="ExternalInput")
    W34_d = nc.dram_tensor("W34", (64, 64), F32, kind="ExternalInput")
    b1_d = nc.dram_tensor("b1b", (128, 64), F32, kind="ExternalInput")
    b2_d = nc.dram_tensor("b2b", (128, 64), F32, kind="ExternalInput")
    b34_d = nc.dram_tensor("b34b", (128, 64), F32, kind="ExternalInput")
    iota_d = nc.dram_tensor("iota", (128, 256), BF, kind="ExternalInput")
    id128_d = nc.dram_tensor("id128", (128, 128), F32, kind="ExternalInput")
    id64_d = nc.dram_tensor("id64", (64, 64), BF, kind="ExternalInput")
    out_d = nc.dram_tensor("out34", (NLOC, 64), F32, kind="ExternalOutput")

    Wp = W + (W & 1)
    TA = (Wp // 2 + 1) // 2
    TB = Wp // 2 - TA
    RA, RB = 128 * TA, 128 * TB
    tablA = [nc.dram_tensor(f"tablA{l}", (RA, 128), BF, kind="Internal")
             for l in range(3)]
    split = TB > 0
    tablB = [nc.dram_tensor(f"tablB{l}", (RB, 128), BF, kind="Internal")
             for l in range(3)] if split else None
    tabfA = [nc.dram_tensor(f"tabfA{l}", (N_CORES * RA, 128), BF, kind="Internal",
                            addr_space="Shared") for l in range(3)]
    tabfB = [nc.dram_tensor(f"tabfB{l}", (N_CORES * RB, 128), BF, kind="Internal",
                            addr_space="Shared") for l in range(3)] if split else None

    AG = mybir.AluOpType
    RG = [list(range(N_CORES))]

    with tile.TileContext(nc) as tc:
        with (
            tc.tile_pool(name="const", bufs=1) as const,
            tc.tile_pool(name="big", bufs=1) as big,
            tc.tile_pool(name="tt", bufs=1) as ttp,
            tc.tile_pool(name="work", bufs=4) as work,
            tc.tile_pool(name="gp", bufs=int(os.environ.get("K_GP","8"))) as gp,
            tc.tile_pool(name="sp", bufs=8) as sp,
            tc.tile_pool(name="psT", bufs=2, space="PSUM") as psT,
            tc.tile_pool(name="psR", bufs=2, space="PSUM") as psR,
            tc.tile_pool(name="psA", bufs=int(os.environ.get("K_PSA","4")), space="PSUM") as psA,
        ):
            # ---- constant loads ----
            def cload(dram, shape, dt, tag):
                t = const.tile(shape, dt, tag=tag)
                nc.sync.dma_start(t[:], dram[:])
                return t

            idxs = cload(idxs_d, [128, P_tot // 16], I16, "idxs")
            dsel = cload(dsel_d, [128, P_tot // 128], F32, "dsel")
            iota = cload(iota_d, [128, 256], BF, "iota")
            id128 = cload(id128_d, [128, 128], F32, "id128")
            id64 = cload(id64_d, [64, 64], BF, "id64")
            W1t = cload(W1_d, [128, 64], F32, "W1t")
            W2t = cload(W2_d, [64, 64], F32, "W2t")
            W34t = cload(W34_d, [64, 64], F32, "W34t")
            b1t = cload(b1_d, [128, 64], F32, "b1t")
            b2t = cload(b2_d, [128, 64], F32, "b2t")
            b34t = cload(b34_d, [128, 64], F32, "b34t")

            NCH = (NLOC + 511) // 512
            hT2 = [big.tile([64, 512], F32, name=f"hT2_{j}", tag=f"hT2_{j}")
                   for j in range(NCH)]
            hT3 = [big.tile([64, 512], F32, name=f"hT3_{j}", tag=f"hT3_{j}")
                   for j in range(NCH)]

            degp = const.tile([128, W], F32, tag="degp")
            nc.sync.dma_start(degp[:], degp_d[:])
            sqp = const.tile([128, W], F32, tag="sqp")
            nc.scalar.activation(sqp[:], degp[:], mybir.ActivationFunctionType.Sqrt)
            dinvp = const.tile([128, W], F32, tag="dinvp")
            nc.vector.reciprocal(dinvp[:], sqp[:])

            CP = int(os.environ.get("K_CP", "0"))

            def nw_cols(nw):
                return 64

            def transform(l, hT, K, Wt):
                """T^T = W^T @ hT, bf16 [64, NLOC] (dinv applied in build_table)."""
                TT = ttp.tile([64, NLOC], BF, tag="TT")
                for c0 in range(0, NLOC, 512):
                    cn = min(512, NLOC - c0)
                    if hT is None:
                        xc = work.tile([128, 512], F32, tag="xc")
                        nc.sync.dma_start(xc[:, :cn], xT_d[:, c0:c0 + cn])
                        rhs = xc[:K, :cn]
                    else:
                        rhs = hT[c0 // 512][:K, :cn]
                    ps = psT.tile([64, 512], F32, tag="psT")
                    nc.tensor.matmul(ps[:, :cn], Wt[:K, :], rhs,
                                     start=True, stop=True)
                    if CP & 2:
                        nc.vector.tensor_copy(TT[:, c0:c0 + cn], ps[:, :cn])
                    else:
                        nc.scalar.copy(TT[:, c0:c0 + cn], ps[:, :cn])
                return TT

            def build_table(l, TT):
                stage = ttp.tile([128, Wp * 64], BF, tag="stage")
                if W != Wp:
                    nc.vector.memset(stage[:, W * 64:], 0.0)
                for w in range(W):
                    c0 = 128 * w
                    nw = min(128, NLOC - c0)
                    ptt = psR.tile([128, 128], BF, tag="ptr")
                    pt = ptt[:, :64]
                    nc.tensor.transpose(pt[:nw, :], TT[:, c0:c0 + nw], id64[:, :])
                    if nw < 128:
                        nc.vector.memset(stage[:, 64 * w:64 * (w + 1)], 0.0)
                    nc.scalar.activation(
                        stage[:nw, 64 * w:64 * w + 64], pt[:nw, :],
                        mybir.ActivationFunctionType.Copy,
                        scale=dinvp[:nw, w:w + 1])
                CA = 2 * TA * 64
                nc.sync.dma_start(
                    tablA[l][:].rearrange("(p r) e -> p (r e)", p=128),
                    stage[:, :CA])
                if split:
                    nc.sync.dma_start(
                        tablB[l][:].rearrange("(p r) e -> p (r e)", p=128),
                        stage[:, CA:])
                if not solo:
                    nc.gpsimd.collective_compute(
                        "AllGather", AG.bypass, replica_groups=RG,
                        ins=[tablA[l][:].opt()], outs=[tabfA[l][:].opt()])
                    if split:
                        nc.gpsimd.collective_compute(
                            "AllGather", AG.bypass, replica_groups=RG,
                            ins=[tablB[l][:].opt()], outs=[tabfB[l][:].opt()])

            def aggregate(l, bias_t, relu, hT_next):
                partial = big.tile([128, W * 64], F32, name=f"partA{l}",
                                   tag="partA")

                def seg_pass(is_b):
                    last = is_b or not split
                    tabsrc = (tabfB if is_b else tabfA)[l]
                    nts = NTB if is_b else NTA
                    cums = cumB if is_b else cumA
                    pws = P_Bs if is_b else P_As
                    for w0 in range(0, W, GRP):
                        wn = min(GRP, W - w0)
                        ntg = sum(nts[w0:w0 + wn])
                        pg = int(cums[w0 + wn] - cums[w0])
                        soff = int(cums[w0]) // 16
                        g = gp.tile([128, GRP * NT_MAX, 128], BF, tag="g")
                        nc.gpsimd.dma_gather(
                            g[:, :ntg, :], tabsrc[:],
                            idxs[:, soff: soff + pg // 16],
                            pg, pg, 128, single_packet=False)
                        tb = 0
                        for w in range(w0, w0 + wn):
                            c0 = 128 * w
                            nw = min(128, NLOC - c0)
                            nt = nts[w]
                            toff = int(cums[w]) // 128
                            ps = psA.tile([128, 64], F32, tag="psA")
                            for t in range(nt):
                                S = sp.tile([128, 256], BF, tag="S")
                                nc.vector.tensor_scalar(
                                    out=S[:], in0=iota[:],
                                    scalar1=dsel[:, toff + t: toff + t + 1],
                                    scalar2=None, op0=AG.is_equal)
                                nc.tensor.matmul(
                                    ps[:], S[:, 0:128], g[:, tb + t, 0:64],
                                    start=(t == 0), stop=False)
                                nc.tensor.matmul(
                                    ps[:], S[:, 128:256], g[:, tb + t, 64:128],
                                    start=False, stop=(t == nt - 1))
                            tb += nt
                            if not last:
                                if CP & 1:
                                    nc.vector.tensor_copy(
                                        partial[:, 64 * w:64 * w + 64], ps[:])
                                else:
                                    nc.scalar.copy(
                                        partial[:, 64 * w:64 * w + 64], ps[:])
                                continue
                            # last pass epilogue: combine, scale, bias, relu
                            hw_ = work.tile([128, 64], F32, tag="hw")
                            if split:
                                nc.vector.tensor_tensor(
                                    out=hw_[:], in0=ps[:],
                                    in1=partial[:, 64 * w:64 * w + 64],
                                    op=AG.add)
                            else:
                                nc.scalar.copy(hw_[:], ps[:])
                            if CP & 8:
                                nc.vector.tensor_scalar(
                                    out=hw_[:], in0=hw_[:],
                                    scalar1=dinvp[:, w:w + 1],
                                    scalar2=None, op0=AG.mult)
                            else:
                                nc.scalar.activation(
                                    hw_[:], hw_[:],
                                    mybir.ActivationFunctionType.Copy,
                                    scale=dinvp[:, w:w + 1])
                            nc.vector.tensor_tensor(
                                out=hw_[:], in0=hw_[:], in1=bias_t[:], op=AG.add)
                            if relu:
                                nc.scalar.activation(
                                    hw_[:], hw_[:],
                                    mybir.ActivationFunctionType.Relu)
                            if hT_next is not None:
                                pt = psR.tile([64, 128], F32, tag="ptr")
                                nc.tensor.transpose(pt[:, :nw], hw_[:nw, :],
                                                    id128[:nw, :nw])
                                j, r0 = c0 // 512, c0 % 512
                                if CP & 4:
                                    nc.vector.tensor_copy(
                                        hT_next[j][:, r0:r0 + nw], pt[:, :nw])
                                else:
                                    nc.scalar.copy(hT_next[j][:, r0:r0 + nw],
                                                   pt[:, :nw])
                            else:
                                nc.sync.dma_start(out_d[c0:c0 + nw, :],
                                                  hw_[:nw, :])

                seg_pass(False)
                if split:
                    seg_pass(True)

            PH = int(os.environ.get("K_PHASES", "9"))
            ONLY_AGG = os.environ.get("K_ONLY_AGG", "0") == "1"
            if ONLY_AGG:
                for _l in range(3):
                    aggregate(0, b1t, True, hT2)
                PH = 0
            REP = int(os.environ.get("K_REPEAT", "1"))
            for _rep in range(REP):
                if PH >= 1:
                    TT = transform(0, None, 128, W1t)
                if PH >= 2:
                    build_table(0, TT)
                if PH >= 3:
                    aggregate(0, b1t, True, hT2)
                if PH >= 4:
                    TT = transform(1, hT2, 64, W2t)
                if PH >= 5:
                    build_table(1, TT)
                if PH >= 6:
                    aggregate(1, b2t, True, hT3)
                if PH >= 7:
                    TT = transform(2, hT3, 64, W34t)
                if PH >= 8:
                    build_table(2, TT)
                if PH >= 9:
                    aggregate(2, b34t, False, None)

    nc.compile()
    return nc


def _run(inputs, N, E):
    NLOC = N // N_CORES
    W = (NLOC + 127) // 128

    x = np.asarray(inputs["x"], np.float32)
    ei = np.asarray(inputs["edge_index"], np.int64)
    W1 = np.asarray(inputs["W1"], np.float32)
    b1 = np.asarray(inputs["b1"], np.float32)
    W2 = np.asarray(inputs["W2"], np.float32)
    b2 = np.asarray(inputs["b2"], np.float32)
    Wmu = np.asarray(inputs["Wmu"], np.float32)
    bmu = np.asarray(inputs["bmu"], np.float32)
    Wls = np.asarray(inputs["Wls"], np.float32)
    bls = np.asarray(inputs["bls"], np.float32)

    loop = np.arange(N, dtype=np.int64)
    src = np.concatenate([ei[0], loop])
    dst = np.concatenate([ei[1], loop])
    deg = np.bincount(dst, minlength=N).astype(np.float32)

    # balanced window assignment (equalizes per-window counts across cores)
    lpos = _balance(deg, N, NLOC, W)

    idx_tiles, dsel_tiles, P_ws, P_tot = _prep_edges(src, dst, N, NLOC, W, lpos)

    key = (N, NLOC, W, tuple(P_ws[0]), tuple(P_ws[1]),
           os.environ.get("K_PHASES", "9"), os.environ.get("K_REPEAT", "1"),
           os.environ.get("K_SOLO", "0"), os.environ.get("K_ONLY_AGG", "0"),
           os.environ.get("K_CP", "0"), os.environ.get("K_GRP", "4"))
    if key not in _cache:
        _cache[key] = _build(N, NLOC, W, P_ws)
    nc = _cache[key]

    W34 = np.concatenate([Wmu, Wls], axis=1)
    b34 = np.concatenate([bmu, bls])
    iota = np.ascontiguousarray(np.tile(np.arange(256, dtype=np.float32),
                                        (128, 1))).astype(BF16)
    id128 = np.eye(128, dtype=np.float32)
    id64 = np.eye(64, dtype=np.float32).astype(BF16)
    b1b = np.ascontiguousarray(np.tile(b1, (128, 1)))
    b2b = np.ascontiguousarray(np.tile(b2, (128, 1)))
    b34b = np.ascontiguousarray(np.tile(b34, (128, 1)))

    in_maps = []
    for c in range(N_CORES):
        degc = deg[c * NLOC:(c + 1) * NLOC]
        lc = lpos[c * NLOC:(c + 1) * NLOC]
        degp = np.ones(W * 128, np.float32)
        degp[lc] = degc
        xp = np.empty((NLOC, x.shape[1]), np.float32)
        xp[lc] = x[c * NLOC:(c + 1) * NLOC]
        in_maps.append({
            "xT": np.ascontiguousarray(xp.T),
            "idxs": idx_tiles[c],
            "dsel": dsel_tiles[c],
            "degp": np.ascontiguousarray(degp.reshape(W, 128).T),
            "W1": W1, "W2": W2, "W34": W34,
            "b1b": b1b, "b2b": b2b, "b34b": b34b,
            "iota": iota, "id128": id128, "id64": id64,
        })

    _last["nc"] = nc
    _last["in_maps"] = in_maps
    res = run_bass_kernel_spmd(nc, in_maps, core_ids=list(range(N_CORES)))
    out = np.empty((N, 64), np.float32)
    for c in range(N_CORES):
        lc = lpos[c * NLOC:(c + 1) * NLOC]
        out[c * NLOC:(c + 1) * NLOC] = res.results[c]["out34"][lc]
    return out[:, :32].copy(), out[:, 32:].copy()


def kernel(**inputs):
    x = np.asarray(inputs["x"])
    ei = np.asarray(inputs["edge_index"])
    return _run(inputs, x.shape[0], ei.shape[1])


# revision 32
# speedup vs baseline: 1.0448x; 1.0073x over previous
"""3-layer GCN encoder (GCNConv x4, layers 3+4 fused) on 8 Trainium2 NeuronCores.

Strategy (graph/data parallel, matches the edge-cut sharding hint):
  - Nodes are partitioned contiguously across the 8 cores (NLOC = N/8 per core).
  - Each layer: local transform H = h @ W (PE, fp32), rows scaled by dinv[node],
    cast to bf16, packed two-nodes-per-256B-row into a local table slice, then
    AllGather -> full table in each core's HBM.
  - Aggregation: per 128-dst-node window, dma_gather fetches the (src-pair) rows
    for all in-edges (dst-grouped, padded to x128); a one-hot matrix S built on
    DVE (iota-256 is_equal dst_rel + 128*parity) turns segment-sum into PE
    matmuls accumulating in PSUM (node-major [128,64] f32).
  - Epilogue: x dinv[dst], + bias, ReLU (layers 1,2); final layer writes
    [NLOC, 64] = [mu | logstd] to DRAM.
  - Pair packing keeps gather indices = src>>1 < 32768 (int16 limit), halves
    the exchanged table bytes, and the parity select folds into the one-hot.

Self-contained: only needs numpy/ml_dtypes/concourse (container-installed).
"""

import os
import sys

if "/opt/trn_rl_repo" not in sys.path:
    sys.path.insert(0, "/opt/trn_rl_repo")

import numpy as np
import ml_dtypes

import concourse.bass as bass
import concourse.bacc as bacc
import concourse.mybir as mybir
import concourse.tile as tile
from concourse.bass_utils import run_bass_kernel_spmd

BF16 = ml_dtypes.bfloat16
F32 = mybir.dt.float32
BF = mybir.dt.bfloat16
I16 = mybir.dt.int16

N_CORES = 8

_cache = {}
_last = {}


def last_run(trace=False, **kw):
    """Re-run the last compiled kernel/in_maps (optionally with NTFF tracing)."""
    if "nc" not in _last:
        return None
    return run_bass_kernel_spmd(_last["nc"], _last["in_maps"],
                                core_ids=list(range(N_CORES)), trace=trace, **kw)


def _balance(deg, N, NLOC, W):
    """Deal degree-sorted nodes round-robin into windows, per core."""
    lpos = np.empty(N, np.int64)
    caps = np.full(W, 128, np.int64)
    caps[W - 1] = NLOC - 128 * (W - 1)
    for c in range(N_CORES):
        dl = deg[c * NLOC:(c + 1) * NLOC]
        order_ = np.argsort(-dl, kind="stable")
        fill = np.zeros(W, np.int64)
        wi = 0
        pos = np.empty(NLOC, np.int64)
        for i in range(NLOC):
            while fill[wi % W] >= caps[wi % W]:
                wi += 1
            ww = wi % W
            pos[order_[i]] = ww * 128 + fill[ww]
            fill[ww] += 1
            wi += 1
        lpos[c * NLOC:(c + 1) * NLOC] = pos
    return lpos


def _prep_edges(src, dst, N, NLOC, W, lpos=None):
    """Group in-edges by (dst core, dst window); pad each window to x128 slots
    uniformly across cores. Returns per-core int16 gather indices / bf16 dsel
    tiles plus per-window padded counts."""
    Wp = W + (W & 1)          # windows padded to even (pair = adjacent windows)
    TA = (Wp // 2 + 1) // 2   # chunk-A t-range (src windows [0, 2*TA))
    TB = Wp // 2 - TA         # chunk-B t-range
    RA, RB = 128 * TA, 128 * TB
    if lpos is None:
        lpos = np.arange(len(np.empty(0)))  # placeholder
        lpos = np.arange(N, dtype=np.int64) % NLOC
    core = dst // NLOC
    local = lpos[dst]
    w = local >> 7
    rel = local & 127
    # source -> (core, partition slot, window) -> chunk / row / parity
    sc = src // NLOC
    sl = lpos[src]
    sp = sl & 127
    sw = sl >> 7
    st = sw >> 1
    spar = sw & 1
    isB = st >= TA
    srow = np.where(isB, sc * RB + sp * TB + (st - TA),
                    sc * RA + sp * TA + st)

    # group edges by (core, dst-window, chunk)
    key = (core * W + w) * 2 + isB
    order = np.argsort(key, kind="stable")
    ksort = key[order]
    counts = np.bincount(key, minlength=N_CORES * W * 2).reshape(N_CORES, W, 2)
    P_As = np.maximum((counts[:, :, 0].max(0) + 127) // 128 * 128, 128)
    if TB > 0:
        P_Bs = np.maximum((counts[:, :, 1].max(0) + 127) // 128 * 128, 128)
    else:
        P_Bs = np.zeros(W, np.int64)
    PA_tot = int(P_As.sum())
    # slot layout: [all A segments by window][all B segments by window]
    cumA = np.concatenate([[0], np.cumsum(P_As)])
    cumB = np.concatenate([[0], np.cumsum(P_Bs)]) + PA_tot
    P_tot = int(PA_tot + P_Bs.sum())
    gstart = np.concatenate([[0], np.cumsum(counts.reshape(-1))])
    pos_in_group = np.arange(len(ksort)) - gstart[ksort]
    w_of = (ksort >> 1) % W
    c_of = (ksort >> 1) // W
    b_of = ksort & 1
    slot = np.where(b_of == 1, cumB[w_of], cumA[w_of]) + pos_in_group

    idx_arr = np.zeros((N_CORES, P_tot), np.int16)
    dsel_arr = np.full((N_CORES, P_tot), 300.0, np.float32)
    idx_arr[c_of, slot] = srow[order].astype(np.int16)
    dsel_arr[c_of, slot] = rel[order] + 128.0 * spar[order]

    idx_tiles = []
    dsel_tiles = []
    for c in range(N_CORES):
        idx16 = np.ascontiguousarray(idx_arr[c].reshape(P_tot // 16, 16).T)
        idx_tiles.append(np.ascontiguousarray(np.tile(idx16, (8, 1))))
        dsel_tiles.append(
            np.ascontiguousarray(dsel_arr[c].reshape(P_tot // 128, 128).T)
        )
    return idx_tiles, dsel_tiles, (list(map(int, P_As)), list(map(int, P_Bs))), P_tot


def _build(N, NLOC, W, P_ABs):
    """Build the 8-core SPMD Bass program. Returns compiled nc."""
    P_As, P_Bs = [list(map(int, p)) for p in P_ABs]
    PA_tot = sum(P_As)
    P_tot = PA_tot + sum(P_Bs)
    NTA = [p // 128 for p in P_As]
    NTB = [p // 128 for p in P_Bs]
    NT_MAX = max(NTA + NTB)
    cumA = np.concatenate([[0], np.cumsum(P_As)]).astype(int)
    cumB = (np.concatenate([[0], np.cumsum(P_Bs)]) + PA_tot).astype(int)
    GRP = int(os.environ.get("K_GRP", "1"))  # windows per gather instruction

    solo = os.environ.get("K_SOLO", "0") == "1"
    nc = bacc.Bacc("TRN2", target_bir_lowering=False, debug=False,
                   num_devices=1 if solo else N_CORES)

    xT_d = nc.dram_tensor("xT", (128, NLOC), F32, kind="ExternalInput")
    idxs_d = nc.dram_tensor("idxs", (128, P_tot // 16), I16, kind="ExternalInput")
    dsel_d = nc.dram_tensor("dsel", (128, P_tot // 128), F32, kind="ExternalInput")
    degp_d = nc.dram_tensor("degp", (128, W), F32, kind="ExternalInput")
    W1_d = nc.dram_tensor("W1", (128, 64), F32, kind="ExternalInput")
    W2_d = nc.dram_tensor("W2", (64, 64), F32, kind="ExternalInput")
    W34_d = nc.dram_tensor("W34", (64, 64), F32, kind="ExternalInput")
    b1_d = nc.dram_tensor("b1b", (128, 64), F32, kind="ExternalInput")
    b2_d = nc.dram_tensor("b2b", (128, 64), F32, kind="ExternalInput")
    b34_d = nc.dram_tensor("b34b", (128, 64), F32, kind="ExternalInput")
    iota_d = nc.dram_tensor("iota", (128, 256), BF, kind="ExternalInput")
    id128_d = nc.dram_tensor("id128", (128, 128), F32, kind="ExternalInput")
    id64_d = nc.dram_tensor("id64", (64, 64), BF, kind="ExternalInput")
    out_d = nc.dram_tensor("out34", (NLOC, 64), F32, kind="ExternalOutput")

    Wp = W + (W & 1)
    TA = (Wp // 2 + 1) // 2
    TB = Wp // 2 - TA
    RA, RB = 128 * TA, 128 * TB
    tablA = [nc.dram_tensor(f"tablA{l}", (RA, 128), BF, kind="Internal")
             for l in range(3)]
    split = TB > 0
    tablB = [nc.dram_tensor(f"tablB{l}", (RB, 128), BF, kind="Internal")
             for l in range(3)] if split else None
    tabfA = [nc.dram_tensor(f"tabfA{l}", (N_CORES * RA, 128), BF, kind="Internal",
                            addr_space="Shared") for l in range(3)]
    tabfB = [nc.dram_tensor(f"tabfB{l}", (N_CORES * RB, 128), BF, kind="Internal",
                            addr_space="Shared") for l in range(3)] if split else None

    AG = mybir.AluOpType
    RG = [list(range(N_CORES))]

    with tile.TileContext(nc) as tc:
        with (
            tc.tile_pool(name="const", bufs=1) as const,
            tc.tile_pool(name="big", bufs=1) as big,
            tc.tile_pool(name="tt", bufs=int(os.environ.get("K_TT","2"))) as ttp,
            tc.tile_pool(name="work", bufs=int(os.environ.get("K_WK","8"))) as work,
            tc.tile_pool(name="gp", bufs=int(os.environ.get("K_GP","8"))) as gp,
            tc.tile_pool(name="sp", bufs=int(os.environ.get("K_SB","8"))) as sp,
            tc.tile_pool(name="psT", bufs=2, space="PSUM") as psT,
            tc.tile_pool(name="psR", bufs=2, space="PSUM") as psR,
            tc.tile_pool(name="psA", bufs=int(os.environ.get("K_PSA","4")), space="PSUM") as psA,
        ):
            # ---- constant loads ----
            def cload(dram, shape, dt, tag):
                t = const.tile(shape, dt, tag=tag)
                nc.sync.dma_start(t[:], dram[:])
                return t

            idxs = cload(idxs_d, [128, P_tot // 16], I16, "idxs")
            dsel = cload(dsel_d, [128, P_tot // 128], F32, "dsel")
            iota = cload(iota_d, [128, 256], BF, "iota")
            id128 = cload(id128_d, [128, 128], F32, "id128")
            id64 = cload(id64_d, [64, 64], BF, "id64")
            W1t = cload(W1_d, [128, 64], F32, "W1t")
            W2t = cload(W2_d, [64, 64], F32, "W2t")
            W34t = cload(W34_d, [64, 64], F32, "W34t")
            b1t = cload(b1_d, [128, 64], F32, "b1t")
            b2t = cload(b2_d, [128, 64], F32, "b2t")
            b34t = cload(b34_d, [128, 64], F32, "b34t")

            NCH = (NLOC + 511) // 512
            hT2 = [big.tile([64, 512], F32, name=f"hT2_{j}", tag=f"hT2_{j}")
                   for j in range(NCH)]
            hT3 = [big.tile([64, 512], F32, name=f"hT3_{j}", tag=f"hT3_{j}")
                   for j in range(NCH)]

            degp = const.tile([128, W], F32, tag="degp")
            nc.sync.dma_start(degp[:], degp_d[:])
            sqp = const.tile([128, W], F32, tag="sqp")
            nc.scalar.activation(sqp[:], degp[:], mybir.ActivationFunctionType.Sqrt)
            dinvp = const.tile([128, W], F32, tag="dinvp")
            nc.vector.reciprocal(dinvp[:], sqp[:])

            CP = int(os.environ.get("K_CP", "0"))

            def nw_cols(nw):
                return 64

            def transform(l, hT, K, Wt):
                """T^T = W^T @ hT, bf16 [64, NLOC] (dinv applied in build_table)."""
                TT = ttp.tile([64, NLOC], BF, tag="TT")
                for c0 in range(0, NLOC, 512):
                    cn = min(512, NLOC - c0)
                    if hT is None:
                        xc = work.tile([128, 512], F32, tag="xc")
                        nc.sync.dma_start(xc[:, :cn], xT_d[:, c0:c0 + cn])
                        rhs = xc[:K, :cn]
                    else:
                        rhs = hT[c0 // 512][:K, :cn]
                    ps = psT.tile([64, 512], F32, tag="psT")
                    nc.tensor.matmul(ps[:, :cn], Wt[:K, :], rhs,
                                     start=True, stop=True)
                    if CP & 2:
                        nc.vector.tensor_copy(TT[:, c0:c0 + cn], ps[:, :cn])
                    else:
                        nc.scalar.copy(TT[:, c0:c0 + cn], ps[:, :cn])
                return TT

            def build_table(l, TT):
                stage = ttp.tile([128, Wp * 64], BF, tag="stage")
                if W != Wp:
                    nc.vector.memset(stage[:, W * 64:], 0.0)
                for w in range(W):
                    c0 = 128 * w
                    nw = min(128, NLOC - c0)
                    ptt = psR.tile([128, 128], BF, tag="ptr")
                    pt = ptt[:, :64]
                    nc.tensor.transpose(pt[:nw, :], TT[:, c0:c0 + nw], id64[:, :])
                    if nw < 128:
                        nc.vector.memset(stage[:, 64 * w:64 * (w + 1)], 0.0)
                    nc.scalar.activation(
                        stage[:nw, 64 * w:64 * w + 64], pt[:nw, :],
                        mybir.ActivationFunctionType.Copy,
                        scale=dinvp[:nw, w:w + 1])
                CA = 2 * TA * 64
                nc.sync.dma_start(
                    tablA[l][:].rearrange("(p r) e -> p (r e)", p=128),
                    stage[:, :CA])
                if split:
                    nc.sync.dma_start(
                        tablB[l][:].rearrange("(p r) e -> p (r e)", p=128),
                        stage[:, CA:])
                if not solo:
                    nc.gpsimd.collective_compute(
                        "AllGather", AG.bypass, replica_groups=RG,
                        ins=[tablA[l][:].opt()], outs=[tabfA[l][:].opt()])
                    if split:
                        nc.gpsimd.collective_compute(
                            "AllGather", AG.bypass, replica_groups=RG,
                            ins=[tablB[l][:].opt()], outs=[tabfB[l][:].opt()])

            def aggregate(l, bias_t, relu, hT_next):
                partial = big.tile([128, W * 64], F32, name=f"partA{l}",
                                   tag="partA")

                def seg_pass(is_b):
                    last = is_b or not split
                    tabsrc = (tabfB if is_b else tabfA)[l]
                    nts = NTB if is_b else NTA
                    cums = cumB if is_b else cumA
                    pws = P_Bs if is_b else P_As
                    for w0 in range(0, W, GRP):
                        wn = min(GRP, W - w0)
                        ntg = sum(nts[w0:w0 + wn])
                        pg = int(cums[w0 + wn] - cums[w0])
                        soff = int(cums[w0]) // 16
                        g = gp.tile([128, GRP * NT_MAX, 128], BF, tag="g")
                        nc.gpsimd.dma_gather(
                            g[:, :ntg, :], tabsrc[:],
                            idxs[:, soff: soff + pg // 16],
                            pg, pg, 128, single_packet=False)
                        tb = 0
                        for w in range(w0, w0 + wn):
                            c0 = 128 * w
                            nw = min(128, NLOC - c0)
                            nt = nts[w]
                            toff = int(cums[w]) // 128
                            ps = psA.tile([128, 64], F32, tag="psA")
                            for t in range(nt):
                                S = sp.tile([128, 256], BF, tag="S")
                                nc.vector.tensor_scalar(
                                    out=S[:], in0=iota[:],
                                    scalar1=dsel[:, toff + t: toff + t + 1],
                                    scalar2=None, op0=AG.is_equal)
                                nc.tensor.matmul(
                                    ps[:], S[:, 0:128], g[:, tb + t, 0:64],
                                    start=(t == 0), stop=False)
                                nc.tensor.matmul(
                                    ps[:], S[:, 128:256], g[:, tb + t, 64:128],
                                    start=False, stop=(t == nt - 1))
                            tb += nt
                            if not last:
                                if CP & 1:
                                    nc.vector.tensor_copy(
                                        partial[:, 64 * w:64 * w + 64], ps[:])
                                else:
                                    nc.scalar.copy(
                                        partial[:, 64 * w:64 * w + 64], ps[:])
                                continue
                            # last pass epilogue: combine, scale, bias, relu
                            hw_ = work.tile([128, 64], F32, tag="hw")
                            if split:
                                nc.vector.tensor_tensor(
                                    out=hw_[:], in0=ps[:],
                                    in1=partial[:, 64 * w:64 * w + 64],
                                    op=AG.add)
                            else:
                                nc.scalar.copy(hw_[:], ps[:])
                            if CP & 8:
                                nc.vector.tensor_scalar(
                                    out=hw_[:], in0=hw_[:],
                                    scalar1=dinvp[:, w:w + 1],
                                    scalar2=None, op0=AG.mult)
                            else:
                                nc.scalar.activation(
                                    hw_[:], hw_[:],
                                    mybir.ActivationFunctionType.Copy,
                                    scale=dinvp[:, w:w + 1])
                            nc.vector.tensor_tensor(
                                out=hw_[:], in0=hw_[:], in1=bias_t[:], op=AG.add)
                            if relu:
                                nc.scalar.activation(
                                    hw_[:], hw_[:],
                                    mybir.ActivationFunctionType.Relu)
                            if hT_next is not None:
                                pt = psR.tile([64, 128], F32, tag="ptr")
                                nc.tensor.transpose(pt[:, :nw], hw_[:nw, :],
                                                    id128[:nw, :nw])
                                j, r0 = c0 // 512, c0 % 512
                                if CP & 4:
                                    nc.vector.tensor_copy(
                                        hT_next[j][:, r0:r0 + nw], pt[:, :nw])
                                else:
                                    nc.scalar.copy(hT_next[j][:, r0:r0 + nw],
                                                   pt[:, :nw])
                            else:
                                nc.sync.dma_start(out_d[c0:c0 + nw, :],
                                                  hw_[:nw, :])

                seg_pass(False)
                if split:
                    seg_pass(True)

            PH = int(os.environ.get("K_PHASES", "9"))
            ONLY_AGG = os.environ.get("K_ONLY_AGG", "0") == "1"
            if ONLY_AGG:
                for _l in range(3):
                    aggregate(0, b1t, True, hT2)
                PH = 0
            REP = int(os.environ.get("K_REPEAT", "1"))
            for _rep in range(REP):
                if PH >= 1:
                    TT = transform(0, None, 128, W1t)
                if PH >= 2:
                    build_table(0, TT)
                if PH >= 3:
                    aggregate(0, b1t, True, hT2)
                if PH >= 4:
                    TT = transform(1, hT2, 64, W2t)
                if PH >= 5:
                    build_table(1, TT)
                if PH >= 6:
                    aggregate(1, b2t, True, hT3)
                if PH >= 7:
                    TT = transform(2, hT3, 64, W34t)
                if PH >= 8:
                    build_table(2, TT)
                if PH >= 9:
                    aggregate(2, b34t, False, None)

    nc.compile()
    return nc


def _run(inputs, N, E):
    NLOC = N // N_CORES
    W = (NLOC + 127) // 128

    x = np.asarray(inputs["x"], np.float32)
    ei = np.asarray(inputs["edge_index"], np.int64)
    W1 = np.asarray(inputs["W1"], np.float32)
    b1 = np.asarray(inputs["b1"], np.float32)
    W2 = np.asarray(inputs["W2"], np.float32)
    b2 = np.asarray(inputs["b2"], np.float32)
    Wmu = np.asarray(inputs["Wmu"], np.float32)
    bmu = np.asarray(inputs["bmu"], np.float32)
    Wls = np.asarray(inputs["Wls"], np.float32)
    bls = np.asarray(inputs["bls"], np.float32)

    loop = np.arange(N, dtype=np.int64)
    src = np.concatenate([ei[0], loop])
    dst = np.concatenate([ei[1], loop])
    deg = np.bincount(dst, minlength=N).astype(np.float32)

    # balanced window assignment (equalizes per-window counts across cores)
    lpos = _balance(deg, N, NLOC, W)

    idx_tiles, dsel_tiles, P_ws, P_tot = _prep_edges(src, dst, N, NLOC, W, lpos)

    key = (N, NLOC, W, tuple(P_ws[0]), tuple(P_ws[1]),
           os.environ.get("K_PHASES", "9"), os.environ.get("K_REPEAT", "1"),
           os.environ.get("K_SOLO", "0"), os.environ.get("K_ONLY_AGG", "0"),
           os.environ.get("K_CP", "0"), os.environ.get("K_GRP", "4"))
    if key not in _cache:
        _cache[key] = _build(N, NLOC, W, P_ws)
    nc = _cache[key]

    W34 = np.concatenate([Wmu, Wls], axis=1)
    b34 = np.concatenate([bmu, bls])
    iota = np.ascontiguousarray(np.tile(np.arange(256, dtype=np.float32),
                                        (128, 1))).astype(BF16)
    id128 = np.eye(128, dtype=np.float32)
    id64 = np.eye(64, dtype=np.float32).astype(BF16)
    b1b = np.ascontiguousarray(np.tile(b1, (128, 1)))
    b2b = np.ascontiguousarray(np.tile(b2, (128, 1)))
    b34b = np.ascontiguousarray(np.tile(b34, (128, 1)))

    in_maps = []
    for c in range(N_CORES):
        degc = deg[c * NLOC:(c + 1) * NLOC]
        lc = lpos[c * NLOC:(c + 1) * NLOC]
        degp = np.ones(W * 128, np.float32)
        degp[lc] = degc
        xp = np.empty((NLOC, x.shape[1]), np.float32)
        xp[lc] = x[c * NLOC:(c + 1) * NLOC]
        in_maps.append({
            "xT": np.ascontiguousarray(xp.T),
            "idxs": idx_tiles[c],
            "dsel": dsel_tiles[c],
            "degp": np.ascontiguousarray(degp.reshape(W, 128).T),
            "W1": W1, "W2": W2, "W34": W34,
            "b1b": b1b, "b2b": b2b, "b34b": b34b,
            "iota": iota, "id128": id128, "id64": id64,
        })

    _last["nc"] = nc
    _last["in_maps"] = in_maps
    res = run_bass_kernel_spmd(nc, in_maps, core_ids=list(range(N_CORES)))
    out = np.empty((N, 64), np.float32)
    for c in range(N_CORES):
        lc = lpos[c * NLOC:(c + 1) * NLOC]
        out[c * NLOC:(c + 1) * NLOC] = res.results[c]["out34"][lc]
    return out[:, :32].copy(), out[:, 32:].copy()


def kernel(**inputs):
    x = np.asarray(inputs["x"])
    ei = np.asarray(inputs["edge_index"])
    return _run(inputs, x.shape[0], ei.shape[1])
